# revision 56
# baseline (speedup 1.0000x reference)
"""DynamicFocus attention kernel for Trainium2 (8 NeuronCores, Bass/Tile).

Sharding: tensor-parallel over heads (2 heads/core) through the attention;
ReduceScatter of the output-projection partials, then sequence-parallel
residual + LayerNorm (256 query rows/core).

Per head:
  ip  = qh @ kh^T
  s   = w0/8*ip + w1*ip/(|q_i||k_j|) - w2*sqrt(|q_i|^2 + |k_j|^2 - 2 ip)
  attn = softmax(s);  ctx = attn @ vh;  out = LN(ctx @ Wo + bo + q)

Device formulation (everything on one ACT table set, natural_log_exp):
  s1 PSUM  = [w0'*qp ; w1*rq*qp]^T . [kp ; rk*kp]          (contraction 128)
  d2w PSUM = [-2 w2^2 qp ; 1]^T . [kp ; w2^2 k2]           (contraction 65)
  dist     = exp(0.5*ln(d2w + (w2^2 q2 + eps)))            (= w2*sqrt(d2))
  s1      -= dist   (PE matmul with -I)
  e        = exp(s1), row sums via ACT accum_out
  attn     = e * (1/sum)  -> DMA out; PE-transpose -> bf16 attnT
  ctxT     = vp^T . attnT (bf16); partial res = ctxT^T @ Wo_slice
  ReduceScatter(add) -> res rows for this core -> +bo +q -> LayerNorm
"""
import sys

if "/opt/trn_rl_repo" not in sys.path:
    sys.path.insert(0, "/opt/trn_rl_repo")

import numpy as np
from contextlib import ExitStack

import concourse.bass as bass
import concourse.tile as tile
from concourse import bacc, mybir
from concourse.bass_utils import run_bass_kernel_spmd
from concourse.masks import make_identity

N_CORES = 8
B, T, D, H = 1, 2048, 1024, 16
HD = D // H          # 64
HPC = H // N_CORES   # 2 heads per core
FS = HPC * HD        # 128 f-columns per core
QS = T // N_CORES    # 256 query rows per core
F32 = mybir.dt.float32
F32R = mybir.dt.float32r
BF16 = mybir.dt.bfloat16
AF = mybir.ActivationFunctionType
ALU = mybir.AluOpType

_CACHE = {}


def _round_f32r(a: np.ndarray) -> np.ndarray:
    """fp32 -> fp32r (mantissa rounded to 11 bits), matching the PE's input
    rounding for float32r matmuls."""
    u = np.ascontiguousarray(a, dtype=np.float32).view(np.uint32)
    r = ((u.astype(np.uint64) + 0x800) & 0xFFFFF000).astype(np.uint32)
    return r.view(np.float32)


def _patch_act_tables():
    """Force every activation onto natural_log_exp_and_others (it contains
    Ln/Exp/Identity/Square) so the table-load pass emits one load instead of
    thrashing between the exp and ln sets (~2.7us per reload)."""
    import concourse.bacc as _bacc_mod
    import concourse.hw_specs as _hw
    if getattr(_bacc_mod, "_act_tables_patched", False):
        return
    _orig = _hw.get_activation_tables
    keep = "natural_log_exp_and_others"

    def _patched(arch):
        t = _orig(arch)
        return {name: (fns if name == keep else set())
                for name, fns in t.items()}

    _bacc_mod.get_activation_tables = _patched
    _bacc_mod._act_tables_patched = True


def _build(sim_single=False):
    _patch_act_tables()
    nc = bacc.Bacc("TRN2", target_bir_lowering=False, debug=False,
                   num_devices=1 if sim_single else N_CORES)

    # ---- DRAM parameters (per-core slices supplied by the host) ----
    qT = nc.dram_tensor("qT", [D, T], F32R, kind="ExternalInput").ap()
    kT = nc.dram_tensor("kT", [D, T], F32R, kind="ExternalInput").ap()
    vT = nc.dram_tensor("vT", [D, T], F32R, kind="ExternalInput").ap()
    wq = nc.dram_tensor("wq", [D, FS], F32R, kind="ExternalInput").ap()
    wk = nc.dram_tensor("wk", [D, FS], F32R, kind="ExternalInput").ap()
    wv = nc.dram_tensor("wv", [D, FS], F32R, kind="ExternalInput").ap()
    wo = nc.dram_tensor("wo", [FS, D], F32R, kind="ExternalInput").ap()
    bq = nc.dram_tensor("bq", [FS], F32, kind="ExternalInput").ap()
    bk = nc.dram_tensor("bk", [FS], F32, kind="ExternalInput").ap()
    bv = nc.dram_tensor("bv", [FS], F32, kind="ExternalInput").ap()
    bo = nc.dram_tensor("bo", [D], F32, kind="ExternalInput").ap()
    gamma = nc.dram_tensor("gamma", [D], F32, kind="ExternalInput").ap()
    beta = nc.dram_tensor("beta", [D], F32, kind="ExternalInput").ap()
    qres = nc.dram_tensor("qres", [QS, D], F32, kind="ExternalInput").ap()
    # consts: [w0p, w1, neg2w2sq, w2sq, epsd, ln_eps, unused, unused]
    consts = nc.dram_tensor("consts", [8], F32, kind="ExternalInput").ap()

    attn_out = nc.dram_tensor("attn_out", [HPC, T, T], F32,
                              kind="ExternalOutput").ap()
    normed_out = nc.dram_tensor("normed_out", [QS, D], F32,
                                kind="ExternalOutput").ap()

    pres = nc.dram_tensor("pres", [T, D], F32)          # partial res (this core)
    res_slice = nc.dram_tensor("res_slice", [QS, D], F32)  # after ReduceScatter

    with tile.TileContext(nc) as tc, ExitStack() as top:
        const_p = top.enter_context(tc.tile_pool(name="const", bufs=1))

        # one packed const tile: cols 0-7 = broadcast consts, 8/9/10 = bq/bk/bv
        c_all = const_p.tile([128, 16], F32)
        for j in range(6):
            nc.sync.dma_start(out=c_all[:, j:j + 1],
                              in_=consts[j:j + 1].to_broadcast([128, 1]))
        nc.sync.dma_start(out=c_all[:, 8:9], in_=bq.rearrange("(f o) -> f o", o=1))
        nc.sync.dma_start(out=c_all[:, 9:10], in_=bk.rearrange("(f o) -> f o", o=1))
        nc.sync.dma_start(out=c_all[:, 10:11], in_=bv.rearrange("(f o) -> f o", o=1))
        w0p_c = c_all[0:64, 0:1]
        w1_c = c_all[:, 1:2]
        neg2w2sq_c = c_all[0:64, 2:3]
        w2sq_c128 = c_all[:, 3:4]
        epsd_c = c_all[:, 4:5]
        lneps_c = c_all[:, 5:6]
        bq_t = c_all[:, 8:9]
        bk_t = c_all[:, 9:10]
        bv_t = c_all[:, 10:11]

        # packed f32 block: slot 0 = identity, slot 1 = all-ones (for rows)
        f32pk = const_p.tile([128, 2, 128], F32)
        make_identity(nc, f32pk[:, 0, :])
        nc.vector.memset(f32pk[:, 1, :], 1.0)
        ident = f32pk[:, 0, :]
        # packed f32r block: slot 0 = identity, slot 1 = -identity, slot 2 col0=ones
        f32rpk = const_p.tile([128, 3, 128], F32R)
        nc.vector.tensor_copy(f32rpk[:, 0, :], ident)
        nc.vector.tensor_scalar_mul(f32rpk[:, 1, :], ident, -1.0)
        nc.vector.memset(f32rpk[:, 2, :].bitcast(F32), 1.0)
        ident_r = f32rpk[:, 0, :]
        negI_r = f32rpk[:, 1, :]
        ones_col = f32pk[0:64, 1, 0:1]

        # pools reserve space for their whole open window; nest lifetimes.
        stats_p = top.enter_context(tc.tile_pool(name="stats", bufs=1))
        vp_p = top.enter_context(tc.tile_pool(name="vp", bufs=1))
        vp_r = vp_p.tile([128, 16, FS], F32R)        # vp[t%128, tc, f]
        ctx_p = top.enter_context(tc.tile_pool(name="ctxsb", bufs=1))
        asm_stack = ExitStack()
        _asm_p = asm_stack.enter_context(tc.tile_pool(name="asm", bufs=1))
        rows_stack = ExitStack()
        _rows_p = rows_stack.enter_context(tc.tile_pool(name="rows", bufs=1))

        # ---- Phase 1: projections qpT/kpT/vpT ----
        proj_stack = ExitStack()
        proj_p = proj_stack.enter_context(tc.tile_pool(name="proj", bufs=1))
        qpT = proj_p.tile([FS, T], F32)
        kpT = proj_p.tile([FS, T], F32)

        with tc.tile_pool(name="wproj", bufs=1) as wproj_p, \
             tc.tile_pool(name="p1psum", bufs=1, space="PSUM") as pp, \
             tc.tile_pool(name="p1sb", bufs=2) as p1sb:
            w_aps = {"wq": wq, "wk": wk, "wv": wv}
            w_tiles = {}
            for name, src, dst, bias_t in (
                    ("wq", qT, qpT, bq_t), ("wk", kT, kpT, bk_t),
                    ("wv", vT, None, bv_t)):
                wt = wproj_p.tile([128, 8, FS], F32R, tag="w", name=name,
                                  bufs=2)
                nc.sync.dma_start(out=wt,
                                  in_=w_aps[name].rearrange("(c p) f -> p c f",
                                                            p=128))
                w_tiles[name] = wt
                ps_tiles = [pp.tile([FS, 512], F32, tag=f"pj{j}", name=f"pj{j}") for j in range(4)]
                for dc in range(8):
                    src_t = p1sb.tile([128, T], F32R, tag="src", bufs=4)
                    (nc.sync, nc.scalar)[dc % 2].dma_start(
                        out=src_t, in_=src[dc * 128:(dc + 1) * 128, :])
                    for j in range(4):
                        nc.tensor.matmul(ps_tiles[j][:], w_tiles[name][:, dc, :],
                                         src_t[:, j * 512:(j + 1) * 512],
                                         start=(dc == 0), stop=(dc == 7))
                if dst is not None:
                    for j in range(4):
                        nc.scalar.activation(dst[:, j * 512:(j + 1) * 512],
                                             ps_tiles[j][:], AF.Identity,
                                             bias=bias_t)
                else:
                    vpT_sb = p1sb.tile([FS, T], F32R, tag="vpT", bufs=1)
                    for j in range(4):
                        nc.scalar.activation(vpT_sb[:, j * 512:(j + 1) * 512],
                                             ps_tiles[j][:], AF.Identity,
                                             bias=bias_t)
                    for tc_i in range(16):
                        tp = pp.tile([128, 128], F32R, tag="vtr")
                        nc.tensor.transpose(
                            tp[:], vpT_sb[:, tc_i * 128:(tc_i + 1) * 128],
                            ident_r)
                        nc.vector.tensor_copy(vp_r[:, tc_i, :], tp[:])

        # ---- Phase 1b: per-head stats ----
        q2w_all = stats_p.tile([128, 2, 16], F32, name="q2w_all")
        q2w_eps = [q2w_all[:, h, :] for h in range(HPC)]
        rows_p = _rows_p
        rq_row = [rows_p.tile([1, T], F32R, tag=f"rq{h}", name=f"rq{h}")
                  for h in range(HPC)]
        rk_row = [rows_p.tile([1, T], F32R, tag=f"rk{h}", name=f"rk{h}")
                  for h in range(HPC)]
        k2w_row = [rows_p.tile([1, T], F32R, tag=f"k2w{h}", name=f"k2w{h}")
                   for h in range(HPC)]

        with tc.tile_pool(name="p1bpsum", bufs=1, space="PSUM") as sp, \
             tc.tile_pool(name="p1bsb", bufs=1) as ssb:
            for h in range(HPC):
                r0 = h * HD
                sqq = ssb.tile([HD, T], F32, tag="sqq")
                sqk = ssb.tile([HD, T], F32, tag="sqk")
                nc.scalar.activation(sqq[:], qpT[r0:r0 + HD, :], AF.Square)
                nc.scalar.activation(sqk[:], kpT[r0:r0 + HD, :], AF.Square)
                # bias columns: q2 in [128, 16] layout (per-partition bias)
                q2p = sp.tile([128, 16], F32, tag="q2p")
                for tc_i in range(16):
                    sl = slice(tc_i * 128, (tc_i + 1) * 128)
                    nc.tensor.matmul(q2p[:, tc_i:tc_i + 1], sqq[:, sl],
                                     ones_col, start=True, stop=True)
                nc.vector.tensor_scalar(q2w_eps[h], q2p[:], w2sq_c128,
                                        None, ALU.mult)
                nc.vector.tensor_scalar(q2w_eps[h], q2w_eps[h], epsd_c,
                                        None, ALU.add)
                # row stats: q2/k2 as [1, T] psum rows (ones.T @ sq)
                for src_sq, dst_ln in ((sqq, rq_row[h]), (sqk, rk_row[h])):
                    x2row = sp.tile([1, T], F32, tag="x2row", name="x2row")
                    for j in range(4):
                        nc.tensor.matmul(x2row[:, j * 512:(j + 1) * 512],
                                         ones_col, src_sq[:, j * 512:(j + 1) * 512],
                                         start=True, stop=True)
                    if dst_ln is rk_row[h]:
                        # also k2w row = w2^2 * k2  (f32r, feeds KdT row 64)
                        nc.vector.tensor_scalar(k2w_row[h][:], x2row[:],
                                                c_all[0:1, 3:4], None, ALU.mult)
                    lnx = ssb.tile([1, T], F32, tag="lnx", name="lnx")
                    nc.scalar.activation(lnx[:], x2row[:], AF.Ln)
                    nc.scalar.activation(dst_ln[:], lnx[:], AF.Exp, scale=-0.5)
                # fold w1 into rq
                nc.vector.tensor_scalar(rq_row[h][:], rq_row[h][:],
                                        c_all[0:1, 1:2], None, ALU.mult)

        # ---- Phase 1c: assemble score-matmul operands ----
        asm_p = _asm_p
        QsT = [asm_p.tile([128, T], F32R, tag=f"QsT{h}", name=f"QsT{h}") for h in range(HPC)]
        KsT = [asm_p.tile([128, T], F32R, tag=f"KsT{h}", name=f"KsT{h}") for h in range(HPC)]
        QdT = [asm_p.tile([65, T], F32R, tag=f"QdT{h}", name=f"QdT{h}") for h in range(HPC)]
        KdT = [asm_p.tile([65, T], F32R, tag=f"KdT{h}", name=f"KdT{h}") for h in range(HPC)]

        with tc.tile_pool(name="p1cpsum", bufs=2, space="PSUM") as bp:
            for h in range(HPC):
                r0 = h * HD
                nc.vector.tensor_scalar(QsT[h][0:HD, :], qpT[r0:r0 + HD, :],
                                        w0p_c, None, ALU.mult)
                nc.vector.tensor_copy(KsT[h][0:HD, :], kpT[r0:r0 + HD, :])
                nc.vector.tensor_scalar(QdT[h][0:HD, :], qpT[r0:r0 + HD, :],
                                        neg2w2sq_c, None, ALU.mult)
                nc.vector.memset(QdT[h][HD:HD + 1, :].bitcast(F32), 1.0)
                nc.vector.tensor_copy(KdT[h][0:HD, :], kpT[r0:r0 + HD, :])
                nc.sync.dma_start(out=KdT[h][HD:HD + 1, :], in_=k2w_row[h][:])
                for quarter in range(4):
                    sl = slice(quarter * 512, (quarter + 1) * 512)
                    bcq = bp.tile([HD, 512], F32, tag="bcq")
                    nc.tensor.matmul(bcq[:], f32rpk[0:1, 2, 0:64], rq_row[h][:, sl],
                                     start=True, stop=True)
                    nc.vector.tensor_mul(QsT[h][HD:2 * HD, sl],
                                         qpT[r0:r0 + HD, sl], bcq[:])
                    bck = bp.tile([HD, 512], F32, tag="bck")
                    nc.tensor.matmul(bck[:], f32rpk[0:1, 2, 0:64], rk_row[h][:, sl],
                                     start=True, stop=True)
                    nc.vector.tensor_mul(KsT[h][HD:2 * HD, sl],
                                         kpT[r0:r0 + HD, sl], bck[:])
        proj_stack.close()
        rows_stack.close()

        # ---- Phase 2: scores, softmax, attn out, attnT, ctxT ----
        ctxT_sb = ctx_p.tile([FS, T], F32R)
        p2_stack = ExitStack()
        p2sb = p2_stack.enter_context(tc.tile_pool(name="p2sb", bufs=2))
        attnT_p = p2_stack.enter_context(tc.tile_pool(name="attnT", bufs=1))
        attnT = attnT_p.tile([128, 16, 512], F32R)

        with tc.tile_pool(name="p2psum", bufs=1, space="PSUM") as p2p, \
             tc.tile_pool(name="p2psum2", bufs=1, space="PSUM") as p2p2:
            for h in range(HPC):
                for qq in range(4):             # q quarters of 512 rows
                    for qt in range(4):         # q tiles of 128 within quarter
                        q0 = qq * 512 + qt * 128
                        qcol = qq * 4 + qt
                        s1 = p2p.tile([128, 1024], F32, tag="s1")
                        d2 = p2p.tile([128, 1024], F32, tag="d2")
                        e_sb = p2sb.tile([128, T], F32, tag="e", bufs=3)
                        smol = p2sb.tile([128, 4], F32, tag="smol")
                        accs = smol[:, 0:2]
                        for kc2 in range(2):    # k chunks of 1024
                            k0 = kc2 * 1024
                            for j in range(2):
                                ksl = slice(k0 + j * 512, k0 + (j + 1) * 512)
                                psl = slice(j * 512, (j + 1) * 512)
                                nc.tensor.matmul(s1[:, psl],
                                                 QsT[h][:, q0:q0 + 128],
                                                 KsT[h][:, ksl],
                                                 start=True, stop=False)
                                nc.tensor.matmul(d2[:, psl],
                                                 QdT[h][:, q0:q0 + 128],
                                                 KdT[h][:, ksl],
                                                 start=True, stop=True)
                            lnd = p2sb.tile([128, 1024], F32, tag="lnd", bufs=2)
                            nc.scalar.activation(
                                lnd[:], d2[:], AF.Ln,
                                bias=q2w_eps[h][:, qcol:qcol + 1])
                            dist = p2sb.tile([128, 1024], F32R, tag="dist", bufs=2)
                            nc.scalar.activation(dist[:], lnd[:], AF.Exp,
                                                 scale=0.5)
                            for j in range(2):
                                psl = slice(j * 512, (j + 1) * 512)
                                nc.tensor.matmul(s1[:, psl], negI_r,
                                                 dist[:, psl],
                                                 start=False, stop=True)
                            nc.scalar.activation(e_sb[:, k0:k0 + 1024], s1[:],
                                                 AF.Exp,
                                                 accum_out=accs[:, kc2:kc2 + 1])
                        sums = smol[:, 2:3]
                        nc.vector.tensor_add(sums, accs[:, 0:1], accs[:, 1:2])
                        rinv = smol[:, 3:4]
                        nc.vector.reciprocal(rinv, sums)
                        attn_sb = p2sb.tile([128, T], F32R, tag="attn", bufs=3)
                        nc.vector.tensor_scalar(attn_sb[:], e_sb[:],
                                                rinv, None, ALU.mult)
                        dma_eng = (nc.sync, nc.scalar)[qt % 2]
                        dma_eng.dma_start(out=attn_out[h, q0:q0 + 128, :],
                                          in_=attn_sb[:].bitcast(F32))
                        for g in range(2):      # transpose groups of 8 k-tiles
                            tp = p2p2.tile([128, 8, 128], F32R, tag="tp")
                            for kk in range(8):
                                kc = g * 8 + kk
                                nc.tensor.transpose(
                                    tp[:, kk, :],
                                    attn_sb[:, kc * 128:(kc + 1) * 128],
                                    ident_r)
                            nc.vector.tensor_copy(
                                attnT[:, g * 8:(g + 1) * 8,
                                      qt * 128:(qt + 1) * 128], tp[:])
                    # ctx^T for this (head, q-quarter): [64, 512] valid rows
                    ctxp = p2p.tile([128, 512], F32, tag="ctxp")
                    for kc in range(16):
                        nc.tensor.matmul(ctxp[:], vp_r[:, kc, :],
                                         attnT[:, kc, :],
                                         start=(kc == 0), stop=(kc == 15))
                    r0 = h * HD
                    nc.vector.tensor_copy(
                        ctxT_sb[r0:r0 + HD, qq * 512:(qq + 1) * 512],
                        ctxp[r0:r0 + HD, :])

        p2_stack.close()
        asm_stack.close()

        # ---- Phase 2b: partial out-proj + ReduceScatter ----
        with tc.tile_pool(name="p2bpsum", bufs=2, space="PSUM") as rp, \
             tc.tile_pool(name="p2bsb", bufs=3) as rsb:
            wo_t = rsb.tile([FS, D], F32R, tag="wo", name="wo_t", bufs=1)
            nc.sync.dma_start(out=wo_t, in_=wo)
            for qt16 in range(16):
                part = rsb.tile([128, D], F32, tag="part")
                for nt in range(2):
                    ps = rp.tile([128, 512], F32, tag="respart")
                    nc.tensor.matmul(ps[:],
                                     ctxT_sb[:, qt16 * 128:(qt16 + 1) * 128],
                                     wo_t[:, nt * 512:(nt + 1) * 512],
                                     start=True, stop=True)
                    nc.vector.tensor_copy(part[:, nt * 512:(nt + 1) * 512], ps[:])
                (nc.sync, nc.scalar)[qt16 % 2].dma_start(
                    out=pres.ap()[qt16 * 128:(qt16 + 1) * 128, :], in_=part[:])
        if sim_single:
            nc.sync.dma_start(out=res_slice.ap(), in_=pres.ap()[0:QS, :])
        else:
            nc.gpsimd.collective_compute(
                "ReduceScatter", ALU.add,
                replica_groups=[list(range(N_CORES))],
                ins=[pres.ap()], outs=[res_slice.ap()],
            )

        # ---- Phase 3: residual + LayerNorm on this core's q rows ----
        p3sb = top.enter_context(tc.tile_pool(name="p3sb", bufs=2))
        def row_bcast(name, src):
            t = const_p.tile([128, D], F32, tag=name, name=name)
            src_bc = bass.AP(tensor=src.tensor, offset=src.offset,
                             ap=[[0, 128]] + [list(x) for x in src.ap])
            nc.sync.dma_start(out=t, in_=src_bc)
            return t

        gam_bc = row_bcast("gam_bc", gamma)
        bet_bc = row_bcast("bet_bc", beta)
        bo_bc = row_bcast("bo_bc", bo)

        for mt in range(2):
            res_sb = p3sb.tile([128, D], F32, tag="res")
            nc.sync.dma_start(out=res_sb,
                              in_=res_slice.ap()[mt * 128:(mt + 1) * 128, :])
            qres_t = p3sb.tile([128, D], F32, tag="qrest")
            nc.sync.dma_start(out=qres_t, in_=qres[mt * 128:(mt + 1) * 128, :])
            nc.vector.tensor_add(res_sb[:], res_sb[:], bo_bc[:])
            nc.vector.tensor_add(res_sb[:], res_sb[:], qres_t[:])
            st = p3sb.tile([128, 2, 6], F32, tag="bnst")
            for sg in range(2):
                nc.vector.bn_stats(st[:, sg, :],
                                   res_sb[:, sg * 512:(sg + 1) * 512])
            sm3 = p3sb.tile([128, 4], F32, tag="sm3")
            mv = sm3[:, 0:2]
            nc.vector.bn_aggr(mv, st[:])
            # rstd = exp(-0.5 * ln(var + eps))
            lnv = sm3[:, 2:3]
            nc.scalar.activation(lnv, mv[:, 1:2], AF.Ln, bias=lneps_c)
            rstd = sm3[:, 3:4]
            nc.scalar.activation(rstd, lnv, AF.Exp, scale=-0.5)
            nrm = p3sb.tile([128, D], F32, tag="nrm")
            nc.vector.tensor_scalar(nrm[:], res_sb[:], mv[:, 0:1], None,
                                    ALU.subtract)
            nc.vector.tensor_scalar(nrm[:], nrm[:], rstd, None,
                                    ALU.mult)
            nc.vector.tensor_mul(nrm[:], nrm[:], gam_bc[:])
            nc.vector.tensor_add(nrm[:], nrm[:], bet_bc[:])
            nc.sync.dma_start(out=normed_out[mt * 128:(mt + 1) * 128, :],
                              in_=nrm[:])

    nc.compile()
    return nc


def kernel(q, k, v, Wq, bq, Wk, bk, Wv, bv, Wo, bo,
           score_weights, ln_gamma, ln_beta):
    q = np.asarray(q, dtype=np.float32)
    k = np.asarray(k, dtype=np.float32)
    v = np.asarray(v, dtype=np.float32)
    Wq = np.asarray(Wq, dtype=np.float32)
    Wk = np.asarray(Wk, dtype=np.float32)
    Wv = np.asarray(Wv, dtype=np.float32)
    Wo = np.asarray(Wo, dtype=np.float32)
    sw = np.asarray(score_weights, dtype=np.float32)

    if "nc" not in _CACHE:
        _CACHE["nc"] = _build()
    nc = _CACHE["nc"]

    e = np.exp(sw - sw.max())
    w = (e / e.sum()).astype(np.float32)
    w0p = np.float32(w[0] / np.float32(np.sqrt(HD)))
    w2sq = np.float32(w[2] * w[2])
    consts = np.array([w0p, w[1], -2.0 * w2sq, w2sq, w2sq * 1e-12,
                       1e-5, 0.0, 0.0], dtype=np.float32)

    qT = _round_f32r(q[0].T)
    kT = _round_f32r(k[0].T)
    vT = _round_f32r(v[0].T)
    Wq_r, Wk_r, Wv_r, Wo_r = (_round_f32r(x) for x in (Wq, Wk, Wv, Wo))

    in_maps = []
    for c in range(N_CORES):
        fs = slice(c * FS, (c + 1) * FS)
        qs = slice(c * QS, (c + 1) * QS)
        in_maps.append({
            "qT": qT, "kT": kT, "vT": vT,
            "wq": np.ascontiguousarray(Wq_r[:, fs]),
            "wk": np.ascontiguousarray(Wk_r[:, fs]),
            "wv": np.ascontiguousarray(Wv_r[:, fs]),
            "wo": np.ascontiguousarray(Wo_r[fs, :]),
            "bq": np.ascontiguousarray(np.asarray(bq, np.float32)[fs]),
            "bk": np.ascontiguousarray(np.asarray(bk, np.float32)[fs]),
            "bv": np.ascontiguousarray(np.asarray(bv, np.float32)[fs]),
            "bo": np.asarray(bo, np.float32),
            "gamma": np.asarray(ln_gamma, np.float32),
            "beta": np.asarray(ln_beta, np.float32),
            "qres": np.ascontiguousarray(q[0][qs, :]),
            "consts": consts,
        })

    _CACHE["in_maps"] = in_maps
    res = run_bass_kernel_spmd(nc, in_maps, list(range(N_CORES)))

    attn = np.concatenate([res.results[c]["attn_out"] for c in range(N_CORES)],
                          axis=0)[None]          # [1, 16, T, T]
    normed = np.concatenate([res.results[c]["normed_out"]
                             for c in range(N_CORES)], axis=0)[None]  # [1, T, D]
    return normed, attn


# revision 57
# speedup vs baseline: 1.0017x; 1.0017x over previous
"""DynamicFocus attention kernel for Trainium2 (8 NeuronCores, Bass/Tile).

Sharding: tensor-parallel over heads (2 heads/core) through the attention;
ReduceScatter of the output-projection partials, then sequence-parallel
residual + LayerNorm (256 query rows/core).

Per head:
  ip  = qh @ kh^T
  s   = w0/8*ip + w1*ip/(|q_i||k_j|) - w2*sqrt(|q_i|^2 + |k_j|^2 - 2 ip)
  attn = softmax(s);  ctx = attn @ vh;  out = LN(ctx @ Wo + bo + q)

Device formulation (everything on one ACT table set, natural_log_exp):
  s1 PSUM  = [w0'*qp ; w1*rq*qp]^T . [kp ; rk*kp]          (contraction 128)
  d2w PSUM = [-2 w2^2 qp ; 1]^T . [kp ; w2^2 k2]           (contraction 65)
  dist     = exp(0.5*ln(d2w + (w2^2 q2 + eps)))            (= w2*sqrt(d2))
  s1      -= dist   (PE matmul with -I)
  e        = exp(s1), row sums via ACT accum_out
  attn     = e * (1/sum)  -> DMA out; PE-transpose -> bf16 attnT
  ctxT     = vp^T . attnT (bf16); partial res = ctxT^T @ Wo_slice
  ReduceScatter(add) -> res rows for this core -> +bo +q -> LayerNorm
"""
import sys

if "/opt/trn_rl_repo" not in sys.path:
    sys.path.insert(0, "/opt/trn_rl_repo")

import numpy as np
from contextlib import ExitStack

import concourse.bass as bass
import concourse.tile as tile
from concourse import bacc, mybir
from concourse.bass_utils import run_bass_kernel_spmd
from concourse.masks import make_identity

N_CORES = 8
B, T, D, H = 1, 2048, 1024, 16
HD = D // H          # 64
HPC = H // N_CORES   # 2 heads per core
FS = HPC * HD        # 128 f-columns per core
QS = T // N_CORES    # 256 query rows per core
F32 = mybir.dt.float32
F32R = mybir.dt.float32r
BF16 = mybir.dt.bfloat16
AF = mybir.ActivationFunctionType
ALU = mybir.AluOpType

_CACHE = {}


def _round_f32r(a: np.ndarray) -> np.ndarray:
    """fp32 -> fp32r (mantissa rounded to 11 bits), matching the PE's input
    rounding for float32r matmuls."""
    u = np.ascontiguousarray(a, dtype=np.float32).view(np.uint32)
    r = ((u.astype(np.uint64) + 0x800) & 0xFFFFF000).astype(np.uint32)
    return r.view(np.float32)


def _patch_act_tables():
    """Force every activation onto natural_log_exp_and_others (it contains
    Ln/Exp/Identity/Square) so the table-load pass emits one load instead of
    thrashing between the exp and ln sets (~2.7us per reload)."""
    import concourse.bacc as _bacc_mod
    import concourse.hw_specs as _hw
    if getattr(_bacc_mod, "_act_tables_patched", False):
        return
    _orig = _hw.get_activation_tables
    keep = "natural_log_exp_and_others"

    def _patched(arch):
        t = _orig(arch)
        return {name: (fns if name == keep else set())
                for name, fns in t.items()}

    _bacc_mod.get_activation_tables = _patched
    _bacc_mod._act_tables_patched = True


def _build(sim_single=False):
    _patch_act_tables()
    nc = bacc.Bacc("TRN2", target_bir_lowering=False, debug=False,
                   num_devices=1 if sim_single else N_CORES)

    # ---- DRAM parameters (per-core slices supplied by the host) ----
    qT = nc.dram_tensor("qT", [D, T], F32R, kind="ExternalInput").ap()
    kT = nc.dram_tensor("kT", [D, T], F32R, kind="ExternalInput").ap()
    vT = nc.dram_tensor("vT", [D, T], F32R, kind="ExternalInput").ap()
    wq = nc.dram_tensor("wq", [D, FS], F32R, kind="ExternalInput").ap()
    wk = nc.dram_tensor("wk", [D, FS], F32R, kind="ExternalInput").ap()
    wv = nc.dram_tensor("wv", [D, FS], F32R, kind="ExternalInput").ap()
    wo = nc.dram_tensor("wo", [FS, D], F32R, kind="ExternalInput").ap()
    bq = nc.dram_tensor("bq", [FS], F32, kind="ExternalInput").ap()
    bk = nc.dram_tensor("bk", [FS], F32, kind="ExternalInput").ap()
    bv = nc.dram_tensor("bv", [FS], F32, kind="ExternalInput").ap()
    bo = nc.dram_tensor("bo", [D], F32, kind="ExternalInput").ap()
    gamma = nc.dram_tensor("gamma", [D], F32, kind="ExternalInput").ap()
    beta = nc.dram_tensor("beta", [D], F32, kind="ExternalInput").ap()
    qres = nc.dram_tensor("qres", [QS, D], F32, kind="ExternalInput").ap()
    # consts: [w0p, w1, neg2w2sq, w2sq, epsd, ln_eps, unused, unused]
    consts = nc.dram_tensor("consts", [8], F32, kind="ExternalInput").ap()

    attn_out = nc.dram_tensor("attn_out", [HPC, T, T], F32,
                              kind="ExternalOutput").ap()
    normed_out = nc.dram_tensor("normed_out", [QS, D], F32,
                                kind="ExternalOutput").ap()

    pres = nc.dram_tensor("pres", [T, D], F32)          # partial res (this core)
    res_slice = nc.dram_tensor("res_slice", [QS, D], F32)  # after ReduceScatter

    with tile.TileContext(nc) as tc, ExitStack() as top:
        const_p = top.enter_context(tc.tile_pool(name="const", bufs=1))

        # one packed const tile: cols 0-7 = broadcast consts, 8/9/10 = bq/bk/bv
        c_all = const_p.tile([128, 16], F32)
        for j in range(6):
            nc.sync.dma_start(out=c_all[:, j:j + 1],
                              in_=consts[j:j + 1].to_broadcast([128, 1]))
        nc.sync.dma_start(out=c_all[:, 8:9], in_=bq.rearrange("(f o) -> f o", o=1))
        nc.sync.dma_start(out=c_all[:, 9:10], in_=bk.rearrange("(f o) -> f o", o=1))
        nc.sync.dma_start(out=c_all[:, 10:11], in_=bv.rearrange("(f o) -> f o", o=1))
        w0p_c = c_all[0:64, 0:1]
        w1_c = c_all[:, 1:2]
        neg2w2sq_c = c_all[0:64, 2:3]
        w2sq_c128 = c_all[:, 3:4]
        epsd_c = c_all[:, 4:5]
        lneps_c = c_all[:, 5:6]
        bq_t = c_all[:, 8:9]
        bk_t = c_all[:, 9:10]
        bv_t = c_all[:, 10:11]

        # packed f32 block: slot 0 = identity, slot 1 = all-ones (for rows)
        f32pk = const_p.tile([128, 2, 128], F32)
        make_identity(nc, f32pk[:, 0, :])
        nc.vector.memset(f32pk[:, 1, :], 1.0)
        ident = f32pk[:, 0, :]
        # packed f32r block: slot 0 = identity, slot 1 = -identity, slot 2 col0=ones
        f32rpk = const_p.tile([128, 3, 128], F32R)
        nc.vector.tensor_copy(f32rpk[:, 0, :], ident)
        nc.vector.tensor_scalar_mul(f32rpk[:, 1, :], ident, -1.0)
        nc.vector.memset(f32rpk[:, 2, :].bitcast(F32), 1.0)
        ident_r = f32rpk[:, 0, :]
        negI_r = f32rpk[:, 1, :]
        ones_col = f32pk[0:64, 1, 0:1]

        # pools reserve space for their whole open window; nest lifetimes.
        stats_p = top.enter_context(tc.tile_pool(name="stats", bufs=1))
        vp_p = top.enter_context(tc.tile_pool(name="vp", bufs=1))
        vp_r = vp_p.tile([128, 16, FS], F32R)        # vp[t%128, tc, f]
        ctx_p = top.enter_context(tc.tile_pool(name="ctxsb", bufs=1))
        wo_t = ctx_p.tile([FS, D], F32R, tag="wo", name="wo_t")
        nc.sync.dma_start(out=wo_t, in_=wo)
        asm_stack = ExitStack()
        _asm_p = asm_stack.enter_context(tc.tile_pool(name="asm", bufs=1))
        rows_stack = ExitStack()
        _rows_p = rows_stack.enter_context(tc.tile_pool(name="rows", bufs=1))

        # ---- Phase 1: projections qpT/kpT/vpT ----
        proj_stack = ExitStack()
        proj_p = proj_stack.enter_context(tc.tile_pool(name="proj", bufs=1))
        qpT = proj_p.tile([FS, T], F32)
        kpT = proj_p.tile([FS, T], F32)

        with tc.tile_pool(name="wproj", bufs=1) as wproj_p, \
             tc.tile_pool(name="p1psum", bufs=1, space="PSUM") as pp, \
             tc.tile_pool(name="p1sb", bufs=2) as p1sb:
            w_aps = {"wq": wq, "wk": wk, "wv": wv}
            w_tiles = {}
            for name, src, dst, bias_t in (
                    ("wq", qT, qpT, bq_t), ("wk", kT, kpT, bk_t),
                    ("wv", vT, None, bv_t)):
                wt = wproj_p.tile([128, 8, FS], F32R, tag="w", name=name,
                                  bufs=2)
                nc.sync.dma_start(out=wt,
                                  in_=w_aps[name].rearrange("(c p) f -> p c f",
                                                            p=128))
                w_tiles[name] = wt
                ps_tiles = [pp.tile([FS, 512], F32, tag=f"pj{j}", name=f"pj{j}") for j in range(4)]
                for dc in range(8):
                    src_t = p1sb.tile([128, T], F32R, tag="src", bufs=3)
                    (nc.sync, nc.scalar)[dc % 2].dma_start(
                        out=src_t, in_=src[dc * 128:(dc + 1) * 128, :])
                    for j in range(4):
                        nc.tensor.matmul(ps_tiles[j][:], w_tiles[name][:, dc, :],
                                         src_t[:, j * 512:(j + 1) * 512],
                                         start=(dc == 0), stop=(dc == 7))
                if dst is not None:
                    for j in range(4):
                        nc.scalar.activation(dst[:, j * 512:(j + 1) * 512],
                                             ps_tiles[j][:], AF.Identity,
                                             bias=bias_t)
                else:
                    vpT_sb = p1sb.tile([FS, T], F32R, tag="vpT", bufs=1)
                    for j in range(4):
                        nc.scalar.activation(vpT_sb[:, j * 512:(j + 1) * 512],
                                             ps_tiles[j][:], AF.Identity,
                                             bias=bias_t)
                    for tc_i in range(16):
                        tp = pp.tile([128, 128], F32R, tag="vtr")
                        nc.tensor.transpose(
                            tp[:], vpT_sb[:, tc_i * 128:(tc_i + 1) * 128],
                            ident_r)
                        nc.vector.tensor_copy(vp_r[:, tc_i, :], tp[:])

        # ---- Phase 1b: per-head stats ----
        q2w_all = stats_p.tile([128, 2, 16], F32, name="q2w_all")
        q2w_eps = [q2w_all[:, h, :] for h in range(HPC)]
        rows_p = _rows_p
        rq_row = [rows_p.tile([1, T], F32R, tag=f"rq{h}", name=f"rq{h}")
                  for h in range(HPC)]
        rk_row = [rows_p.tile([1, T], F32R, tag=f"rk{h}", name=f"rk{h}")
                  for h in range(HPC)]
        k2w_row = [rows_p.tile([1, T], F32R, tag=f"k2w{h}", name=f"k2w{h}")
                   for h in range(HPC)]

        with tc.tile_pool(name="p1bpsum", bufs=1, space="PSUM") as sp, \
             tc.tile_pool(name="p1bsb", bufs=1) as ssb:
            for h in range(HPC):
                r0 = h * HD
                sqq = ssb.tile([HD, T], F32, tag="sqq")
                sqk = ssb.tile([HD, T], F32, tag="sqk")
                nc.scalar.activation(sqq[:], qpT[r0:r0 + HD, :], AF.Square)
                nc.scalar.activation(sqk[:], kpT[r0:r0 + HD, :], AF.Square)
                # bias columns: q2 in [128, 16] layout (per-partition bias)
                q2p = sp.tile([128, 16], F32, tag="q2p")
                for tc_i in range(16):
                    sl = slice(tc_i * 128, (tc_i + 1) * 128)
                    nc.tensor.matmul(q2p[:, tc_i:tc_i + 1], sqq[:, sl],
                                     ones_col, start=True, stop=True)
                nc.vector.tensor_scalar(q2w_eps[h], q2p[:], w2sq_c128,
                                        None, ALU.mult)
                nc.vector.tensor_scalar(q2w_eps[h], q2w_eps[h], epsd_c,
                                        None, ALU.add)
                # row stats: q2/k2 as [1, T] psum rows (ones.T @ sq)
                for src_sq, dst_ln in ((sqq, rq_row[h]), (sqk, rk_row[h])):
                    x2row = sp.tile([1, T], F32, tag="x2row", name="x2row")
                    for j in range(4):
                        nc.tensor.matmul(x2row[:, j * 512:(j + 1) * 512],
                                         ones_col, src_sq[:, j * 512:(j + 1) * 512],
                                         start=True, stop=True)
                    if dst_ln is rk_row[h]:
                        # also k2w row = w2^2 * k2  (f32r, feeds KdT row 64)
                        nc.vector.tensor_scalar(k2w_row[h][:], x2row[:],
                                                c_all[0:1, 3:4], None, ALU.mult)
                    lnx = ssb.tile([1, T], F32, tag="lnx", name="lnx")
                    nc.scalar.activation(lnx[:], x2row[:], AF.Ln)
                    nc.scalar.activation(dst_ln[:], lnx[:], AF.Exp, scale=-0.5)
                # fold w1 into rq
                nc.vector.tensor_scalar(rq_row[h][:], rq_row[h][:],
                                        c_all[0:1, 1:2], None, ALU.mult)

        # ---- Phase 1c: assemble score-matmul operands ----
        asm_p = _asm_p
        QsT = [asm_p.tile([128, T], F32R, tag=f"QsT{h}", name=f"QsT{h}") for h in range(HPC)]
        KsT = [asm_p.tile([128, T], F32R, tag=f"KsT{h}", name=f"KsT{h}") for h in range(HPC)]
        QdT = [asm_p.tile([65, T], F32R, tag=f"QdT{h}", name=f"QdT{h}") for h in range(HPC)]
        KdT = [asm_p.tile([65, T], F32R, tag=f"KdT{h}", name=f"KdT{h}") for h in range(HPC)]

        with tc.tile_pool(name="p1cpsum", bufs=2, space="PSUM") as bp:
            for h in range(HPC):
                r0 = h * HD
                nc.vector.tensor_scalar(QsT[h][0:HD, :], qpT[r0:r0 + HD, :],
                                        w0p_c, None, ALU.mult)
                nc.vector.tensor_copy(KsT[h][0:HD, :], kpT[r0:r0 + HD, :])
                nc.vector.tensor_scalar(QdT[h][0:HD, :], qpT[r0:r0 + HD, :],
                                        neg2w2sq_c, None, ALU.mult)
                nc.vector.memset(QdT[h][HD:HD + 1, :].bitcast(F32), 1.0)
                nc.vector.tensor_copy(KdT[h][0:HD, :], kpT[r0:r0 + HD, :])
                nc.sync.dma_start(out=KdT[h][HD:HD + 1, :], in_=k2w_row[h][:])
                for quarter in range(4):
                    sl = slice(quarter * 512, (quarter + 1) * 512)
                    bcq = bp.tile([HD, 512], F32, tag="bcq")
                    nc.tensor.matmul(bcq[:], f32rpk[0:1, 2, 0:64], rq_row[h][:, sl],
                                     start=True, stop=True)
                    nc.vector.tensor_mul(QsT[h][HD:2 * HD, sl],
                                         qpT[r0:r0 + HD, sl], bcq[:])
                    bck = bp.tile([HD, 512], F32, tag="bck")
                    nc.tensor.matmul(bck[:], f32rpk[0:1, 2, 0:64], rk_row[h][:, sl],
                                     start=True, stop=True)
                    nc.vector.tensor_mul(KsT[h][HD:2 * HD, sl],
                                         kpT[r0:r0 + HD, sl], bck[:])
        proj_stack.close()
        rows_stack.close()

        # ---- Phase 2: scores, softmax, attn out, attnT, ctxT ----
        ctxT_sb = ctx_p.tile([FS, T], F32R)
        p2_stack = ExitStack()
        p2sb = p2_stack.enter_context(tc.tile_pool(name="p2sb", bufs=2))
        attnT_p = p2_stack.enter_context(tc.tile_pool(name="attnT", bufs=1))
        attnT = attnT_p.tile([128, 16, 512], F32R)

        with tc.tile_pool(name="p2psum", bufs=1, space="PSUM") as p2p, \
             tc.tile_pool(name="p2psum2", bufs=1, space="PSUM") as p2p2:
            for h in range(HPC):
                for qq in range(4):             # q quarters of 512 rows
                    for qt in range(4):         # q tiles of 128 within quarter
                        q0 = qq * 512 + qt * 128
                        qcol = qq * 4 + qt
                        s1 = p2p.tile([128, 1024], F32, tag="s1")
                        d2 = p2p.tile([128, 1024], F32, tag="d2")
                        e_sb = p2sb.tile([128, T], F32, tag="e", bufs=3)
                        smol = p2sb.tile([128, 4], F32, tag="smol")
                        accs = smol[:, 0:2]
                        for kc2 in range(2):    # k chunks of 1024
                            k0 = kc2 * 1024
                            for j in range(2):
                                ksl = slice(k0 + j * 512, k0 + (j + 1) * 512)
                                psl = slice(j * 512, (j + 1) * 512)
                                nc.tensor.matmul(s1[:, psl],
                                                 QsT[h][:, q0:q0 + 128],
                                                 KsT[h][:, ksl],
                                                 start=True, stop=False)
                                nc.tensor.matmul(d2[:, psl],
                                                 QdT[h][:, q0:q0 + 128],
                                                 KdT[h][:, ksl],
                                                 start=True, stop=True)
                            lnd = p2sb.tile([128, 1024], F32, tag="lnd", bufs=2)
                            nc.scalar.activation(
                                lnd[:], d2[:], AF.Ln,
                                bias=q2w_eps[h][:, qcol:qcol + 1])
                            dist = p2sb.tile([128, 1024], F32R, tag="dist", bufs=2)
                            nc.scalar.activation(dist[:], lnd[:], AF.Exp,
                                                 scale=0.5)
                            for j in range(2):
                                psl = slice(j * 512, (j + 1) * 512)
                                nc.tensor.matmul(s1[:, psl], negI_r,
                                                 dist[:, psl],
                                                 start=False, stop=True)
                            nc.scalar.activation(e_sb[:, k0:k0 + 1024], s1[:],
                                                 AF.Exp,
                                                 accum_out=accs[:, kc2:kc2 + 1])
                        sums = smol[:, 2:3]
                        nc.vector.tensor_add(sums, accs[:, 0:1], accs[:, 1:2])
                        rinv = smol[:, 3:4]
                        nc.vector.reciprocal(rinv, sums)
                        attn_sb = p2sb.tile([128, T], F32R, tag="attn", bufs=3)
                        nc.vector.tensor_scalar(attn_sb[:], e_sb[:],
                                                rinv, None, ALU.mult)
                        dma_eng = (nc.sync, nc.scalar)[qt % 2]
                        dma_eng.dma_start(out=attn_out[h, q0:q0 + 128, :],
                                          in_=attn_sb[:].bitcast(F32))
                        for g in range(2):      # transpose groups of 8 k-tiles
                            tp = p2p2.tile([128, 8, 128], F32R, tag="tp")
                            for kk in range(8):
                                kc = g * 8 + kk
                                nc.tensor.transpose(
                                    tp[:, kk, :],
                                    attn_sb[:, kc * 128:(kc + 1) * 128],
                                    ident_r)
                            nc.vector.tensor_copy(
                                attnT[:, g * 8:(g + 1) * 8,
                                      qt * 128:(qt + 1) * 128], tp[:])
                    # ctx^T for this (head, q-quarter): [64, 512] valid rows
                    ctxp = p2p.tile([128, 512], F32, tag="ctxp")
                    for kc in range(16):
                        nc.tensor.matmul(ctxp[:], vp_r[:, kc, :],
                                         attnT[:, kc, :],
                                         start=(kc == 0), stop=(kc == 15))
                    r0 = h * HD
                    nc.vector.tensor_copy(
                        ctxT_sb[r0:r0 + HD, qq * 512:(qq + 1) * 512],
                        ctxp[r0:r0 + HD, :])

        p2_stack.close()
        asm_stack.close()

        # ---- Phase 2b: partial out-proj + ReduceScatter ----
        with tc.tile_pool(name="p2bpsum", bufs=2, space="PSUM") as rp, \
             tc.tile_pool(name="p2bsb", bufs=3) as rsb:
            for qt16 in range(16):
                part = rsb.tile([128, D], F32, tag="part", bufs=4)
                for nt in range(2):
                    ps = rp.tile([128, 512], F32, tag="respart")
                    nc.tensor.matmul(ps[:],
                                     ctxT_sb[:, qt16 * 128:(qt16 + 1) * 128],
                                     wo_t[:, nt * 512:(nt + 1) * 512],
                                     start=True, stop=True)
                    nc.vector.tensor_copy(part[:, nt * 512:(nt + 1) * 512], ps[:])
                (nc.sync, nc.scalar)[qt16 % 2].dma_start(
                    out=pres.ap()[qt16 * 128:(qt16 + 1) * 128, :], in_=part[:])
        if sim_single:
            nc.sync.dma_start(out=res_slice.ap(), in_=pres.ap()[0:QS, :])
        else:
            nc.gpsimd.collective_compute(
                "ReduceScatter", ALU.add,
                replica_groups=[list(range(N_CORES))],
                ins=[pres.ap()], outs=[res_slice.ap()],
            )

        # ---- Phase 3: residual + LayerNorm on this core's q rows ----
        p3sb = top.enter_context(tc.tile_pool(name="p3sb", bufs=2))
        def row_bcast(name, src):
            t = const_p.tile([128, D], F32, tag=name, name=name)
            src_bc = bass.AP(tensor=src.tensor, offset=src.offset,
                             ap=[[0, 128]] + [list(x) for x in src.ap])
            nc.sync.dma_start(out=t, in_=src_bc)
            return t

        gam_bc = row_bcast("gam_bc", gamma)
        bet_bc = row_bcast("bet_bc", beta)
        bo_bc = row_bcast("bo_bc", bo)

        for mt in range(2):
            res_sb = p3sb.tile([128, D], F32, tag="res")
            nc.sync.dma_start(out=res_sb,
                              in_=res_slice.ap()[mt * 128:(mt + 1) * 128, :])
            qres_t = p3sb.tile([128, D], F32, tag="qrest")
            nc.sync.dma_start(out=qres_t, in_=qres[mt * 128:(mt + 1) * 128, :])
            nc.vector.tensor_add(res_sb[:], res_sb[:], bo_bc[:])
            nc.vector.tensor_add(res_sb[:], res_sb[:], qres_t[:])
            st = p3sb.tile([128, 2, 6], F32, tag="bnst")
            for sg in range(2):
                nc.vector.bn_stats(st[:, sg, :],
                                   res_sb[:, sg * 512:(sg + 1) * 512])
            sm3 = p3sb.tile([128, 4], F32, tag="sm3")
            mv = sm3[:, 0:2]
            nc.vector.bn_aggr(mv, st[:])
            # rstd = exp(-0.5 * ln(var + eps))
            lnv = sm3[:, 2:3]
            nc.scalar.activation(lnv, mv[:, 1:2], AF.Ln, bias=lneps_c)
            rstd = sm3[:, 3:4]
            nc.scalar.activation(rstd, lnv, AF.Exp, scale=-0.5)
            nrm = p3sb.tile([128, D], F32, tag="nrm")
            nc.vector.tensor_scalar(nrm[:], res_sb[:], mv[:, 0:1], None,
                                    ALU.subtract)
            nc.vector.tensor_scalar(nrm[:], nrm[:], rstd, None,
                                    ALU.mult)
            nc.vector.tensor_mul(nrm[:], nrm[:], gam_bc[:])
            nc.vector.tensor_add(nrm[:], nrm[:], bet_bc[:])
            nc.sync.dma_start(out=normed_out[mt * 128:(mt + 1) * 128, :],
                              in_=nrm[:])

    nc.compile()
    return nc


def kernel(q, k, v, Wq, bq, Wk, bk, Wv, bv, Wo, bo,
           score_weights, ln_gamma, ln_beta):
    q = np.asarray(q, dtype=np.float32)
    k = np.asarray(k, dtype=np.float32)
    v = np.asarray(v, dtype=np.float32)
    Wq = np.asarray(Wq, dtype=np.float32)
    Wk = np.asarray(Wk, dtype=np.float32)
    Wv = np.asarray(Wv, dtype=np.float32)
    Wo = np.asarray(Wo, dtype=np.float32)
    sw = np.asarray(score_weights, dtype=np.float32)

    if "nc" not in _CACHE:
        _CACHE["nc"] = _build()
    nc = _CACHE["nc"]

    e = np.exp(sw - sw.max())
    w = (e / e.sum()).astype(np.float32)
    w0p = np.float32(w[0] / np.float32(np.sqrt(HD)))
    w2sq = np.float32(w[2] * w[2])
    consts = np.array([w0p, w[1], -2.0 * w2sq, w2sq, w2sq * 1e-12,
                       1e-5, 0.0, 0.0], dtype=np.float32)

    qT = _round_f32r(q[0].T)
    kT = _round_f32r(k[0].T)
    vT = _round_f32r(v[0].T)
    Wq_r, Wk_r, Wv_r, Wo_r = (_round_f32r(x) for x in (Wq, Wk, Wv, Wo))

    in_maps = []
    for c in range(N_CORES):
        fs = slice(c * FS, (c + 1) * FS)
        qs = slice(c * QS, (c + 1) * QS)
        in_maps.append({
            "qT": qT, "kT": kT, "vT": vT,
            "wq": np.ascontiguousarray(Wq_r[:, fs]),
            "wk": np.ascontiguousarray(Wk_r[:, fs]),
            "wv": np.ascontiguousarray(Wv_r[:, fs]),
            "wo": np.ascontiguousarray(Wo_r[fs, :]),
            "bq": np.ascontiguousarray(np.asarray(bq, np.float32)[fs]),
            "bk": np.ascontiguousarray(np.asarray(bk, np.float32)[fs]),
            "bv": np.ascontiguousarray(np.asarray(bv, np.float32)[fs]),
            "bo": np.asarray(bo, np.float32),
            "gamma": np.asarray(ln_gamma, np.float32),
            "beta": np.asarray(ln_beta, np.float32),
            "qres": np.ascontiguousarray(q[0][qs, :]),
            "consts": consts,
        })

    _CACHE["in_maps"] = in_maps
    res = run_bass_kernel_spmd(nc, in_maps, list(range(N_CORES)))

    attn = np.concatenate([res.results[c]["attn_out"] for c in range(N_CORES)],
                          axis=0)[None]          # [1, 16, T, T]
    normed = np.concatenate([res.results[c]["normed_out"]
                             for c in range(N_CORES)], axis=0)[None]  # [1, T, D]
    return normed, attn
